# revision 13
# baseline (speedup 1.0000x reference)
"""DualAttention Trainium2 Bass kernel (8-core data-parallel), v2.

Contract: kernel(**inputs) takes the FULL inputs of nn_DualAttention
(B=1024, L=199, V=50000, D=Dp=128) and returns the full [1024, 128] f32
output, equal to reference.reference(**inputs).

v2 strategy (per core, 128 batch rows; only the LAST attention row is
needed):
 - embeddings staged feature-major per core by the host (the on-device
   batched-gather primitives -- InstDMAGatherAnt and multi-column
   indirect DMA offsets -- are broken in this environment; 400 serial
   single-column SWDGE gathers cost ~450us and dominate, so the gather
   is done host-side and the device streams linear chunks).
 - per chunk: masked mean (DVE reduce), K projection (weights
   stationary), q projection, token-major V (data stationary, zero-padded
   chunk tails for uniform 128-wide tiles), scores computed TOKEN-major
   (stationary kT tiles x moving q column) accumulated in one PSUM tile.
 - V kept in SBUF (no DRAM roundtrip).
 - tail: alpha, f32 PE transposes of scoresT, entmax bisection, attw
   transposes, AV (stationary v tiles), relu + L2-norm.
"""
import sys
sys.path.insert(0, '/opt/trn_rl_repo')

import math
import numpy as np
import ml_dtypes

import concourse.bass as bass
import concourse.bacc as bacc
import concourse.mybir as mybir
import concourse.tile as tile
from concourse.bass_utils import run_bass_kernel_spmd

F32 = mybir.dt.float32
BF16 = mybir.dt.bfloat16
I16 = mybir.dt.int16

B, L, V, D = 1024, 199, 50000, 128
P = L + 1                  # 200 tokens (199 items + mean)
NB = 128                   # batches per core
NCORES = 8
NCHUNK = 8
CB = NB // NCHUNK          # 16 batches per chunk
CCOL = CB * P              # 3200 gathered columns per chunk
CPAD = CCOL + 56           # + zero tail for the last batch's V tileB
KCOL = NCHUNK * CCOL       # 25600 kT columns
TBL_ROWS = 25728           # fixed-size per-core compact item table
N_ITER = 12                # bisection iterations (converged ~1e-5 by 12)
AluOp = mybir.AluOpType
Act = mybir.ActivationFunctionType

_cache = {}
_last_in_maps = None

_COMBINED_SET = "natural_log_exp_and_others"
_OUR_FUNCS = None


def _patched_get_activation_tables(arch):
    """Steer the act-table-load pass to the one set containing every
    function this kernel uses (ln/exp/relu/copy/square), so the ACT engine
    loads its LUT once instead of ping-ponging (~56 loads x 1.3us)."""
    import concourse.hw_specs as _hs
    global _OUR_FUNCS
    if _OUR_FUNCS is None:
        _OUR_FUNCS = {Act.Relu, Act.Ln, Act.Exp, Act.Copy, Act.Square,
                      Act.Identity}
    t = _hs.get_activation_tables(arch)
    out = {}
    for name, fns in t.items():
        out[name] = fns if name == _COMBINED_SET else (fns - _OUR_FUNCS)
    return out


def _build(ba_const: float):
    nc = bacc.Bacc(None, target_bir_lowering=False, debug=False)

    xeT_d = nc.declare_dram_parameter("xeT", [128, NB * P], BF16, isOutput=False)
    peT_d = nc.declare_dram_parameter("peT", [128, NB * P], BF16, isOutput=False)
    mb = nc.declare_dram_parameter("mb", [NB, P], F32, isOutput=False)
    wts = {}
    for w in ("wk0", "wk1", "wv0", "wv1", "wq0", "wq1"):
        wts[w] = nc.declare_dram_parameter(w, [D, D], BF16, isOutput=False)
    wa0 = nc.declare_dram_parameter("wa0", [D, 1], BF16, isOutput=False)
    wa1 = nc.declare_dram_parameter("wa1", [D, 1], BF16, isOutput=False)
    ident = nc.declare_dram_parameter("ident", [128, 128], BF16, isOutput=False)
    identf = nc.declare_dram_parameter("identf", [128, 128], F32, isOutput=False)
    bkq = nc.declare_dram_parameter("bkq", [128, 2], F32, isOutput=False)
    out_d = nc.declare_dram_parameter("out", [NB, D], F32, isOutput=True)

    with tile.TileContext(nc) as tc:
        with (
            tc.tile_pool(name="const", bufs=1) as cpool,
            tc.tile_pool(name="ring", bufs=2) as ring,
            tc.tile_pool(name="big", bufs=1) as big,
            tc.tile_pool(name="ent", bufs=1) as ent,
            tc.tile_pool(name="pk", bufs=2, space="PSUM") as pk,
            tc.tile_pool(name="pv", bufs=2, space="PSUM") as pv,
            tc.tile_pool(name="psc", bufs=1, space="PSUM") as psc,
            tc.tile_pool(name="pm", bufs=2, space="PSUM") as pm,
        ):
            # ---- constants ----
            w_sb = {}
            for w in ("wk0", "wk1", "wv0", "wv1", "wq0", "wq1"):
                w_sb[w] = cpool.tile([D, D], BF16, tag=w, name=w)
                nc.sync.dma_start(out=w_sb[w][:], in_=wts[w][:])
            wa0_sb = cpool.tile([D, 1], BF16, tag="wa0")
            wa1_sb = cpool.tile([D, 1], BF16, tag="wa1")
            nc.sync.dma_start(out=wa0_sb[:], in_=wa0[:])
            nc.sync.dma_start(out=wa1_sb[:], in_=wa1[:])
            id_sb = cpool.tile([128, 128], BF16, tag="ident")
            nc.sync.dma_start(out=id_sb[:], in_=ident[:])
            idf_sb = cpool.tile([128, 128], F32, tag="identf")
            nc.sync.dma_start(out=idf_sb[:], in_=identf[:])
            bkq_sb = cpool.tile([128, 2], F32, tag="bkq")
            nc.sync.dma_start(out=bkq_sb[:], in_=bkq[:])
            mb_sb = cpool.tile([NB, P], F32, tag="mb")
            nc.sync.dma_start(out=mb_sb[:], in_=mb[:])

            kT = big.tile([128, KCOL], BF16, tag="kT")
            v_sb = big.tile([128, 2 * NB, 128], BF16, tag="v")
            qT = ent.tile([128, NB], BF16, tag="qT")
            qlx = ent.tile([128, NB], BF16, tag="qlx")
            qlp = ent.tile([128, NB], BF16, tag="qlp")
            # scoresT accumulator: [token-tile 2][token 128][batch 128]
            sT_ps = psc.tile([128, 2, NB], F32, tag="sT")
            nc.vector.memset(sT_ps[64:128, 1, :], 0.0)

            for c in range(NCHUNK):
                gx = ring.tile([128, CPAD], BF16, tag="gx", bufs=2)
                gp = ring.tile([128, CPAD], BF16, tag="gp", bufs=2)
                nc.sync.dma_start(out=gx[:, 0:CCOL],
                                  in_=xeT_d[:, CCOL * c:CCOL * (c + 1)])
                nc.sync.dma_start(out=gp[:, 0:CCOL],
                                  in_=peT_d[:, CCOL * c:CCOL * (c + 1)])
                nc.vector.memset(gx[:, CCOL:CPAD], 0.0)
                nc.vector.memset(gp[:, CCOL:CPAD], 0.0)

                gx3 = gx[:, 0:CCOL].rearrange("p (b t) -> p b t", b=CB)
                gp3 = gp[:, 0:CCOL].rearrange("p (b t) -> p b t", b=CB)

                # masked mean over the 199 item tokens -> column 199
                mean_f = ring.tile([128, CB], F32, tag="mean", bufs=2)
                nc.vector.tensor_reduce(mean_f[:], gx3[:, :, 0:L],
                                        axis=mybir.AxisListType.X, op=AluOp.add)
                nc.vector.tensor_scalar(out=gx3[:, :, L], in0=mean_f[:],
                                        scalar1=1.0 / L, scalar2=None,
                                        op0=AluOp.mult)

                # stash last-token columns for alpha
                nc.vector.tensor_copy(out=qlx[:, CB * c:CB * (c + 1)],
                                      in_=gx3[:, :, L])
                nc.vector.tensor_copy(out=qlp[:, CB * c:CB * (c + 1)],
                                      in_=gp3[:, :, L])

                # K projection (feature-major, weights stationary)
                for g in range(8):
                    cols = slice(400 * g, 400 * (g + 1))
                    kcols = slice(CCOL * c + 400 * g, CCOL * c + 400 * (g + 1))
                    kps = pk.tile([128, 400], F32, tag="kps")
                    nc.tensor.matmul(kps[:], w_sb["wk0"][:], gx[:, cols],
                                     start=True, stop=False)
                    nc.tensor.matmul(kps[:], w_sb["wk1"][:], gp[:, cols],
                                     start=False, stop=True)
                    nc.scalar.activation(kT[:, kcols], kps[:], Act.Relu,
                                         bias=bkq_sb[:, 0:1])

                # q projection for this chunk's 16 batches (scaled by 1/sqrt(D))
                qps = pk.tile([128, CB], F32, tag="kps", name="qps")
                nc.tensor.matmul(qps[:], w_sb["wq0"][:], gx3[:, :, L],
                                 start=True, stop=False)
                nc.tensor.matmul(qps[:], w_sb["wq1"][:], gp3[:, :, L],
                                 start=False, stop=True)
                nc.scalar.activation(qT[:, CB * c:CB * (c + 1)], qps[:],
                                     Act.Relu, bias=bkq_sb[:, 1:2],
                                     scale=1.0 / math.sqrt(D))

                # scoresT: per batch, stationary kT tiles x moving q column
                for l in range(CB):
                    gb = CB * c + l
                    base = CCOL * c + P * l
                    nc.tensor.matmul(sT_ps[:, 0, gb:gb + 1],
                                     kT[:, base:base + 128],
                                     qT[:, gb:gb + 1], start=True, stop=True)
                    nc.tensor.matmul(sT_ps[0:72, 1, gb:gb + 1],
                                     kT[:, base + 128:base + 200],
                                     qT[:, gb:gb + 1], start=True, stop=True)

                # V projection (token-major, data stationary), 2 batches/bank
                for pr in range(CB // 2):
                    vps = pv.tile([128, 4, 128], F32, tag="vps")
                    for h in range(4):
                        l = 2 * pr + h // 2
                        tcols = slice(P * l + 128 * (h % 2),
                                      P * l + 128 * (h % 2) + 128)
                        nc.tensor.matmul(vps[:, h, :], gx[:, tcols],
                                         w_sb["wv0"][:], start=True, stop=False)
                        nc.tensor.matmul(vps[:, h, :], gp[:, tcols],
                                         w_sb["wv1"][:], start=False, stop=True)
                    vbase = 4 * (CB // 2 * c + pr)
                    if pr % 2 == 0:
                        nc.scalar.activation(v_sb[:, vbase:vbase + 4, :], vps[:],
                                             Act.Relu)
                    else:
                        nc.vector.tensor_scalar(
                            out=v_sb[:, vbase:vbase + 4, :], in0=vps[:],
                            scalar1=0.0, scalar2=None, op0=AluOp.max)

            # ---- alpha ----
            al_ps = pk.tile([128, 1], F32, tag="kps", name="alps")
            nc.tensor.matmul(al_ps[:], qlx[:], wa0_sb[:], start=True, stop=False)
            nc.tensor.matmul(al_ps[:], qlp[:], wa1_sb[:], start=False, stop=True)
            am1 = ent.tile([128, 1], F32, tag="am1")     # alpha-1 = sigmoid(.+ba)
            nc.scalar.activation(am1[:], al_ps[:], Act.Exp, scale=-1.0,
                                 bias=-ba_const)         # e^{-(z+ba)}
            nc.vector.tensor_scalar(out=am1[:], in0=am1[:], scalar1=1.0,
                                    scalar2=None, op0=AluOp.add)
            nc.vector.reciprocal(am1[:], am1[:])
            cexp = ent.tile([128, 1], F32, tag="cexp")   # 1/(alpha-1)
            nc.vector.reciprocal(cexp[:], am1[:])
            thi_off = ent.tile([128, 1], F32, tag="thi")  # (1/P)^(alpha-1)
            nc.scalar.activation(thi_off[:], am1[:], Act.Exp, scale=-math.log(P))

            # ---- scores -> batch-major via f32 PE transposes ----
            sT_sb = ent.tile([128, 2, NB], F32, tag="sTsb")
            nc.scalar.activation(sT_sb[:], sT_ps[:], Act.Copy)
            scores = ent.tile([NB, P], F32, tag="scores")
            for h in range(2):
                tp = pm.tile([128, 128], F32, tag="pm", name=f"st{h}")
                nc.tensor.transpose(tp[:], sT_sb[:, h, :], idf_sb[:])
                n = 128 if h == 0 else P - 128
                nc.scalar.activation(scores[:, 128 * h:128 * h + n],
                                     tp[:, 0:n], Act.Copy)

            # ---- entmax bisection, two 64-batch halves (overlap AV) ----
            Xa = ent.tile([NB, P], F32, tag="Xa")
            nc.vector.tensor_tensor(out=Xa[:], in0=scores[:], in1=mb_sb[:],
                                    op=AluOp.add)
            nc.vector.tensor_scalar(out=Xa[:], in0=Xa[:], scalar1=am1[:],
                                    scalar2=None, op0=AluOp.mult)
            mx = ent.tile([NB, 1], F32, tag="mx")
            nc.vector.tensor_reduce(mx[:], Xa[:], axis=mybir.AxisListType.X,
                                    op=AluOp.max)
            tlo = ent.tile([NB, 1], F32, tag="tlo")
            nc.vector.tensor_scalar(out=tlo[:], in0=mx[:], scalar1=-1.0,
                                    scalar2=None, op0=AluOp.add)
            # dm table: dm_i = (1 - thi_off) * 2^-(i+1)
            dmtab = ent.tile([NB, N_ITER], F32, tag="dmtab")
            nc.vector.tensor_scalar(out=dmtab[:, 0:1], in0=thi_off[:],
                                    scalar1=-0.5, scalar2=0.5,
                                    op0=AluOp.mult, op1=AluOp.add)
            for i in range(1, N_ITER):
                nc.vector.tensor_scalar(out=dmtab[:, i:i + 1],
                                        in0=dmtab[:, i - 1:i], scalar1=0.5,
                                        scalar2=None, op0=AluOp.mult)
            eps = ent.tile([NB, 1], F32, tag="eps")
            nc.vector.memset(eps[:], 1e-24)
            tm = ent.tile([NB, 1], F32, tag="tm")
            z = ent.tile([NB, P], F32, tag="z")
            e = ent.tile([NB, P], F32, tag="e")
            S = ent.tile([NB, 1], F32, tag="S")
            msk = ent.tile([NB, 1], mybir.dt.int32, tag="msk")
            attw = ent.tile([NB, P], BF16, tag="attw")
            attwT = ent.tile([128, 2, NB], BF16, tag="attwT")
            nc.vector.memset(attwT[64:128, 1, :], 0.0)
            attT_ps = psc.tile([128, 128], F32, tag="avps")
            HB = NB // 2
            for hh in range(2):
                hs = slice(HB * hh, HB * (hh + 1))
                for it in range(N_ITER):
                    nc.vector.tensor_scalar(out=tm[hs], in0=tlo[hs],
                                            scalar1=dmtab[hs, it:it + 1],
                                            scalar2=None, op0=AluOp.add)
                    nc.vector.tensor_scalar(out=z[hs], in0=Xa[hs],
                                            scalar1=tm[hs],
                                            scalar2=1e-30, op0=AluOp.subtract,
                                            op1=AluOp.max)
                    nc.scalar.activation(z[hs], z[hs], Act.Ln)
                    nc.scalar.activation(e[hs], z[hs], Act.Exp,
                                         scale=cexp[hs], accum_out=S[hs])
                    nc.vector.tensor_scalar(out=msk[hs], in0=S[hs],
                                            scalar1=1.0, scalar2=None,
                                            op0=AluOp.is_ge)
                    nc.vector.copy_predicated(out=tlo[hs], mask=msk[hs],
                                              data=tm[hs])
                nc.vector.reciprocal(S[hs], S[hs])
                nc.vector.tensor_scalar(out=attw[hs], in0=e[hs],
                                        scalar1=S[hs],
                                        scalar2=None, op0=AluOp.mult)

                # attw^T (token-major) for this half
                t0 = pm.tile([128, HB], BF16, tag="pm", name=f"t0h{hh}")
                nc.tensor.transpose(t0[:], attw[hs, 0:128], id_sb[hs, hs])
                nc.vector.tensor_copy(out=attwT[:, 0, hs], in_=t0[:])
                t1 = pm.tile([72, HB], BF16, tag="pm", name=f"t1h{hh}")
                nc.tensor.transpose(t1[:], attw[hs, 128:200], id_sb[hs, hs])
                nc.vector.tensor_copy(out=attwT[0:72, 1, hs], in_=t1[:])

                # AV for this half -> att^T columns
                for b in range(HB * hh, HB * (hh + 1)):
                    nc.tensor.matmul(attT_ps[:, b:b + 1], v_sb[:, 2 * b, :],
                                     attwT[:, 0, b:b + 1],
                                     start=True, stop=False)
                    nc.tensor.matmul(attT_ps[:, b:b + 1], v_sb[:, 2 * b + 1, :],
                                     attwT[:, 1, b:b + 1],
                                     start=False, stop=True)
            attT_sb = ent.tile([128, 128], BF16, tag="attTs")
            nc.scalar.activation(attT_sb[:], attT_ps[:], Act.Copy)
            att_ps = pm.tile([128, 128], BF16, tag="pm", name="attps")
            nc.tensor.transpose(att_ps[:], attT_sb[:], id_sb[:])
            attR = ent.tile([NB, D], F32, tag="attR")
            nc.scalar.activation(attR[:], att_ps[:], Act.Relu)

            # ---- L2 normalize ----
            sq = ent.tile([NB, D], F32, tag="sq")
            s2 = ent.tile([NB, 1], F32, tag="s2")
            nc.scalar.activation(sq[:], attR[:], Act.Square, accum_out=s2[:])
            nc.scalar.activation(s2[:], s2[:], Act.Ln, bias=eps[:])
            nc.scalar.activation(s2[:], s2[:], Act.Exp, scale=-0.5)
            out_sb = ent.tile([NB, D], F32, tag="out")
            nc.vector.tensor_scalar(out=out_sb[:], in0=attR[:], scalar1=s2[:],
                                    scalar2=None, op0=AluOp.mult)
            nc.sync.dma_start(out=out_d[:], in_=out_sb[:])

    import concourse.bacc as _bacc_mod
    _orig = _bacc_mod.get_activation_tables
    _bacc_mod.get_activation_tables = _patched_get_activation_tables
    try:
        nc.compile()
    finally:
        _bacc_mod.get_activation_tables = _orig
    return nc


def _prep_core(c, x, pos, item_bf, pos_bf):
    """Host-side per-core staging: feature-major embeddings + mask."""
    xs = x[c * NB:(c + 1) * NB].astype(np.int64)          # [128, 199]
    ps = pos[c * NB:(c + 1) * NB].astype(np.int64)        # [128, 200]
    mask0 = xs == 0
    xi = np.where(mask0, V, xs)                           # zeros row for masked
    flat_idx = np.full((NB, P), V, dtype=np.int64)        # col 199 -> zeros row
    flat_idx[:, :L] = xi
    xeT = np.ascontiguousarray(item_bf[flat_idx.reshape(-1)].T)  # [128, 25600]
    peT = np.ascontiguousarray(pos_bf[ps.reshape(-1)].T)         # [128, 25600]

    mb = np.zeros((NB, P), dtype=np.float32)
    mb[:, :L] = np.where(mask0, -1e30, 0.0)
    return {"xeT": xeT, "peT": peT, "mb": mb}


def kernel(x, pos, item_emb, pos_emb, Wq, bq, Wk, bk, Wv, bv, wa, ba):
    x = np.asarray(x)
    pos = np.asarray(pos)
    item_emb = np.asarray(item_emb, dtype=np.float32)
    pos_emb = np.asarray(pos_emb, dtype=np.float32)

    item_bf = np.vstack([item_emb, np.zeros((1, D), np.float32)]).astype(
        ml_dtypes.bfloat16)
    pos_bf = pos_emb.astype(ml_dtypes.bfloat16)

    wb = {}
    for name, W in (("wk", Wk), ("wv", Wv), ("wq", Wq)):
        W = np.asarray(W, np.float32)
        wb[name + "0"] = W[:D].astype(ml_dtypes.bfloat16)
        wb[name + "1"] = W[D:].astype(ml_dtypes.bfloat16)
    wa = np.asarray(wa, np.float32)
    wa0 = wa[:D].astype(ml_dtypes.bfloat16)
    wa1 = wa[D:].astype(ml_dtypes.bfloat16)
    bkq = np.stack([np.asarray(bk, np.float32),
                    np.asarray(bq, np.float32) / math.sqrt(D)], axis=1)
    ba_const = float(np.asarray(ba, np.float32).reshape(-1)[0])
    ident = np.eye(128, dtype=ml_dtypes.bfloat16)
    identf = np.eye(128, dtype=np.float32)

    key = ("k5", ba_const)
    if key not in _cache:
        _cache[key] = _build(ba_const)
    nc = _cache[key]

    shared = {"wa0": wa0, "wa1": wa1, "ident": ident,
              "identf": identf, "bkq": bkq}
    shared.update(wb)
    in_maps = []
    for c in range(NCORES):
        m = dict(shared)
        m.update(_prep_core(c, x, pos, item_bf, pos_bf))
        in_maps.append(m)

    global _last_in_maps
    _last_in_maps = in_maps
    res = run_bass_kernel_spmd(nc, in_maps, core_ids=list(range(NCORES)))
    out = np.concatenate([res.results[c]["out"] for c in range(NCORES)], axis=0)
    return out.astype(np.float32)


if __name__ == "__main__":
    d = np.load('/tmp/inputs.npz')
    inp = {k: d[k] for k in d.files}
    got = kernel(**inp)
    ref = np.load('/tmp/ref_out.npy')
    err = np.abs(got - ref).max() / np.abs(ref).max()
    fro = np.linalg.norm(got - ref) / np.linalg.norm(ref)
    print(f"max_rel={err:.3e} fro_rel={fro:.3e}")
